# revision 7
# baseline (speedup 1.0000x reference)
# Trainium2 Bass kernel for nn_ContrastiveReact (contrastive loss with
# min-cosine-distance against a 50000-entry purchase-embedding table).
#
# Strategy:
#   - purch_embeddings are L2-normalized on the host (cheap: 0.2% of FLOPs),
#     transposed to [D=128, P] layout, padded to 50176 columns and sharded
#     over the 8 cores along P (6272 columns per core).
#   - All 4608 query rows (32*(16+128)) are replicated to every core as a
#     [128, 4608] fp16 transposed operand.
#   - Each core computes dots = embsT.T @ phatT for its P-shard on the PE
#     (fp16 inputs, fp32 PSUM) and reduces max over its P columns:
#       * ScalarE copies half of the PSUM banks to SBUF,
#       * VectorE tensor_tensor_reduce(max, max) consumes one PSUM stream and
#         one SBUF stream at once (2 candidates/cycle) with a fused reduction.
#   - Per-core partial maxima [4608] are gathered on the host; the final
#     (tiny) segmented-sum + log-sum-exp over 32 samples runs on the host as
#     part of unsharding.
import os
import tempfile

import numpy as np

import concourse.bacc as bacc
import concourse.bass as bass
import concourse.dve_ops as dve_ops
import concourse.mybir as mybir
import concourse.tile as tile
from concourse.bass_utils import run_bass_kernel_spmd
from concourse.dve_spec import C0, Spec, Src0, Src1, lower, maxx
from concourse.dve_uop import DveOpSpec

# Problem constants (hardcoded per harness contract).
B, NPOS, NNEG, P, D = 32, 16, 128, 50000, 128
NUM_GROUPS = 8
N_CORES = 8
ROWS = B * (NPOS + NNEG)          # 4608
RT = ROWS // 128                  # 36 row tiles
PC = 6272                         # padded P per core (8 * 6272 = 50176)
N_FULL = 12                       # 12 full 512-wide chunks per core
TAIL = PC - N_FULL * 512          # 128
NEG_INIT = -1e30

_CACHE = {}


def _ref_max_max(in0, in1, c0, c1, c2):
    b = np.maximum(in0.astype(np.float32), in1.astype(np.float32))
    acc = np.maximum(c0, b.reshape(b.shape[0], -1).max(axis=-1, keepdims=True))
    return b, acc


def register_max_max():
    """Author a custom DVE op: out = max(in0, in1); accum = max(s0, max(out)).

    One DVE instruction ingests two fp32 streams (one may be PSUM) at
    1 elem/lane/cycle each — 2 max-candidates per cycle — with the reduction
    fused, which is the key to keeping the PSUM->max pipeline off the
    critical path.
    """
    name = "TENSOR_MAX_MAX_ANT"
    for op in dve_ops.OPS:
        if op.name == name:
            return op
    spec = Spec(body=maxx(Src0, Src1), accum=maxx, accum_init=C0,
                reference=_ref_max_max)
    row = dve_ops._CUSTOM_DVE_ROW_BASE + len(dve_ops.OPS)
    assert row < 0x20
    shas = {}
    for ver in ("v3",):
        tmp = DveOpSpec(name=name, opcode=row, uops=lower(spec, ver=ver),
                        rd1_en=True)
        shas[ver] = tmp.sha(ver)
    op = dve_ops.DveOp(name, spec, subdim=False, uops_sha=shas)
    dve_ops.OPS.append(op)
    dve_ops._SUB_OPCODE_FOR_NAME[name] = row
    dve_ops.CUSTOM_DVE_SPECS[name] = spec
    return op


def build_nc():
    mm_op = register_max_max()
    nc = bacc.Bacc()
    ph = nc.dram_tensor("ph", [128, PC], mybir.dt.float16, kind="ExternalInput")
    em = nc.dram_tensor("em", [128, ROWS], mybir.dt.float16, kind="ExternalInput")
    out = nc.dram_tensor("out", [128, RT], mybir.dt.float32, kind="ExternalOutput")

    with tile.TileContext(nc) as tc:
        with (
            tc.tile_pool(name="singles", bufs=1) as singles,
            tc.tile_pool(name="cp", bufs=6) as cps,
            tc.tile_pool(name="accp", bufs=4) as accp,
            tc.tile_pool(name="scr", bufs=2) as scr,
            tc.tile_pool(name="psum", bufs=3, space="PSUM") as pp,
        ):
            em_sb = singles.tile([128, ROWS], mybir.dt.float16)
            nc.sync.dma_start(out=em_sb, in_=em[:, :])
            ph_sb = singles.tile([128, PC], mybir.dt.float16)
            nc.sync.dma_start(out=ph_sb, in_=ph[:, :])
            out_sb = singles.tile([128, RT], mybir.dt.float32)

            for r in range(RT):
                lhsT = em_sb[:, r * 128:(r + 1) * 128]
                acc = accp.tile([128, 4], mybir.dt.float32, tag="acc")
                # First 3 psum pairs (chunks 0..5) -> ScalarE copies to SBUF.
                cp_tiles = []
                for k in range(3):
                    pt = pp.tile([128, 1024], mybir.dt.float32, tag="mm", bufs=3)
                    base = k * 1024
                    nc.tensor.matmul(pt[:, 0:512], lhsT,
                                     ph_sb[:, base:base + 512],
                                     start=True, stop=True)
                    nc.tensor.matmul(pt[:, 512:1024], lhsT,
                                     ph_sb[:, base + 512:base + 1024],
                                     start=True, stop=True)
                    cp = cps.tile([128, 1024], mybir.dt.float32, tag="cp", bufs=6)
                    nc.scalar.copy(out=cp, in_=pt)
                    cp_tiles.append(cp)
                # Next 3 psum pairs (chunks 6..11) -> fused max-max TTR against
                # the SBUF copies (ingests 2048 candidates per op).
                for k in range(3):
                    pt = pp.tile([128, 1024], mybir.dt.float32, tag="mm", bufs=3)
                    base = 3072 + k * 1024
                    nc.tensor.matmul(pt[:, 0:512], lhsT,
                                     ph_sb[:, base:base + 512],
                                     start=True, stop=True)
                    nc.tensor.matmul(pt[:, 512:1024], lhsT,
                                     ph_sb[:, base + 512:base + 1024],
                                     start=True, stop=True)
                    tscr = scr.tile([128, 1024], mybir.dt.float32,
                                    tag="scr", bufs=2)
                    nc.vector._custom_dve(
                        mm_op, out=tscr, in0=pt, in1=cp_tiles[k],
                        s0=NEG_INIT, accum_out=acc[:, k:k + 1])
                # Tail chunk (128 cols).
                tp = pp.tile([128, TAIL], mybir.dt.float32, tag="tail", bufs=2)
                nc.tensor.matmul(tp, lhsT, ph_sb[:, N_FULL * 512:PC],
                                 start=True, stop=True)
                nc.vector.reduce_max(acc[:, 3:4], tp, axis=mybir.AxisListType.X)
                nc.vector.reduce_max(out_sb[:, r:r + 1], acc,
                                     axis=mybir.AxisListType.X)

            nc.sync.dma_start(out=out[:, :], in_=out_sb)
    nc.compile()
    return nc


def _prep(purch_embeddings, pos_embs, neg_embs):
    purch = np.asarray(purch_embeddings, dtype=np.float32)
    pos = np.asarray(pos_embs, dtype=np.float32)
    neg = np.asarray(neg_embs, dtype=np.float32)

    pnorm = np.sqrt((purch.astype(np.float64) ** 2).sum(axis=1))
    phat = purch / np.maximum(pnorm, 1e-8)[:, None]
    phatT = np.zeros((128, N_CORES * PC), dtype=np.float16)
    phatT[:, :P] = phat.T.astype(np.float16)
    shards = [np.ascontiguousarray(phatT[:, c * PC:(c + 1) * PC])
              for c in range(N_CORES)]

    embs = np.concatenate(
        [pos.reshape(B * NPOS, D), neg.reshape(B * NNEG, D)], axis=0)
    enorm = np.sqrt((embs.astype(np.float64) ** 2).sum(axis=1))
    embsT = np.ascontiguousarray(embs.T.astype(np.float16))
    return shards, embsT, enorm


def run_device(shards, embsT, trace=False):
    if "nc" not in _CACHE:
        _CACHE["nc"] = build_nc()
    nc = _CACHE["nc"]
    in_maps = [{"ph": shards[c], "em": embsT} for c in range(N_CORES)]
    kwargs = {}
    if trace:
        kwargs = dict(trace=True, tmpdir=tempfile.mkdtemp(prefix="ctr_"))
    return run_bass_kernel_spmd(nc, in_maps, core_ids=list(range(N_CORES)),
                                **kwargs)


def _finish(results, enorm, cost_pos, cost_neg, neg_seg_ids):
    # Gather/unshard: global max over cores, then the tiny per-sample loss.
    parts = np.stack([r["out"].T.reshape(ROWS) for r in results])  # [8, 4608]
    M = parts.max(axis=0).astype(np.float64)                       # [4608]

    cos_max = M / np.maximum(enorm, 1e-8)
    min_dist = 1.0 - cos_max
    pos_min = min_dist[:B * NPOS].reshape(B, NPOS)
    neg_min = min_dist[B * NPOS:].reshape(B, NNEG)

    cost_pos = np.asarray(cost_pos, dtype=np.float64)
    cost_neg = np.asarray(cost_neg, dtype=np.float64)
    seg = np.asarray(neg_seg_ids).astype(np.int64)

    positive_value = pos_min.sum(axis=1) + cost_pos                # [B]
    seg_sum = np.zeros((B, NUM_GROUPS), dtype=np.float64)
    np.add.at(seg_sum, (np.arange(B)[:, None], seg), neg_min)
    negative_values = seg_sum + cost_neg                           # [B, G]

    num = np.exp(-positive_value)
    den = np.exp(-negative_values).sum(axis=1)
    losses = -np.log(num / (num + den))
    return np.array(losses.mean(), dtype=np.float32)


def kernel(purch_embeddings, pos_embs, neg_embs, cost_pos, cost_neg,
           neg_seg_ids):
    shards, embsT, enorm = _prep(purch_embeddings, pos_embs, neg_embs)
    results = run_device(shards, embsT, trace=False)
    return _finish(results.results, enorm, cost_pos, cost_neg, neg_seg_ids)


# revision 8
# speedup vs baseline: 27.5146x; 27.5146x over previous
# Trainium2 Bass kernel for nn_ContrastiveReact (contrastive loss with
# min-cosine-distance against a 50000-entry purchase-embedding table).
#
# Strategy:
#   - purch_embeddings are L2-normalized on the host (cheap: 0.2% of FLOPs),
#     transposed to [D=128, P] layout, padded to 50176 columns and sharded
#     over the 8 cores along P (6272 columns per core).
#   - All 4608 query rows (32*(16+128)) are replicated to every core as a
#     [128, 4608] fp16 transposed operand.
#   - Each core computes dots = embsT.T @ phatT for its P-shard on the PE
#     (fp16 inputs, fp32 PSUM) and reduces max over its P columns:
#       * ScalarE copies half of the PSUM banks to SBUF,
#       * VectorE tensor_tensor_reduce(max, max) consumes one PSUM stream and
#         one SBUF stream at once (2 candidates/cycle) with a fused reduction.
#   - Per-core partial maxima [4608] are gathered on the host; the final
#     (tiny) segmented-sum + log-sum-exp over 32 samples runs on the host as
#     part of unsharding.
import os
import tempfile

import numpy as np

import concourse.bacc as bacc
import concourse.bass as bass
import concourse.dve_ops as dve_ops
import concourse.mybir as mybir
import concourse.tile as tile
from concourse.bass_utils import run_bass_kernel_spmd
from concourse.dve_spec import C0, Spec, Src0, Src1, lower, maxx
from concourse.dve_uop import DveOpSpec

# Problem constants (hardcoded per harness contract).
B, NPOS, NNEG, P, D = 32, 16, 128, 50000, 128
NUM_GROUPS = 8
N_CORES = 8
ROWS = B * (NPOS + NNEG)          # 4608
RT = ROWS // 128                  # 36 row tiles
PC = 6272                         # padded P per core (8 * 6272 = 50176)
N_FULL = 12                       # 12 full 512-wide chunks per core
TAIL = PC - N_FULL * 512          # 128
NEG_INIT = -1e30

_CACHE = {}


def _ref_max_max(in0, in1, c0, c1, c2):
    b = np.maximum(in0.astype(np.float32), in1.astype(np.float32))
    acc = np.maximum(c0, b.reshape(b.shape[0], -1).max(axis=-1, keepdims=True))
    return b, acc


def register_max_max():
    """Author a custom DVE op: out = max(in0, in1); accum = max(s0, max(out)).

    One DVE instruction ingests two fp32 streams (one may be PSUM) at
    1 elem/lane/cycle each — 2 max-candidates per cycle — with the reduction
    fused, which is the key to keeping the PSUM->max pipeline off the
    critical path.
    """
    name = "TENSOR_MAX_MAX_ANT"
    for op in dve_ops.OPS:
        if op.name == name:
            return op
    spec = Spec(body=maxx(Src0, Src1), accum=maxx, accum_init=C0,
                reference=_ref_max_max)
    row = dve_ops._CUSTOM_DVE_ROW_BASE + len(dve_ops.OPS)
    assert row < 0x20
    shas = {}
    for ver in ("v3",):
        tmp = DveOpSpec(name=name, opcode=row, uops=lower(spec, ver=ver),
                        rd1_en=True)
        shas[ver] = tmp.sha(ver)
    op = dve_ops.DveOp(name, spec, subdim=False, uops_sha=shas)
    dve_ops.OPS.append(op)
    dve_ops._SUB_OPCODE_FOR_NAME[name] = row
    dve_ops.CUSTOM_DVE_SPECS[name] = spec
    return op


def build_nc():
    mm_op = register_max_max()
    nc = bacc.Bacc()
    ph = nc.dram_tensor("ph", [128, PC], mybir.dt.float16, kind="ExternalInput")
    em = nc.dram_tensor("em", [128, ROWS], mybir.dt.float16, kind="ExternalInput")
    out = nc.dram_tensor("out", [128, RT], mybir.dt.float32, kind="ExternalOutput")

    with tile.TileContext(nc) as tc:
        with (
            tc.tile_pool(name="singles", bufs=1) as singles,
            tc.tile_pool(name="cp", bufs=6) as cps,
            tc.tile_pool(name="accp", bufs=4) as accp,
            tc.tile_pool(name="scr", bufs=2) as scr,
            tc.tile_pool(name="psum", bufs=3, space="PSUM") as pp,
        ):
            # Split the input DMAs so the first row-tile's weights and the
            # first purchase chunks land early and matmuls start sooner.
            em_sb = singles.tile([128, ROWS], mybir.dt.float16)
            nc.sync.dma_start(out=em_sb[:, 0:128], in_=em[:, 0:128])
            nc.sync.dma_start(out=em_sb[:, 128:ROWS], in_=em[:, 128:ROWS])
            ph_sb = singles.tile([128, PC], mybir.dt.float16)
            ph_step = PC // 8
            for s in range(8):
                end = PC if s == 7 else (s + 1) * ph_step
                nc.sync.dma_start(out=ph_sb[:, s * ph_step:end],
                                  in_=ph[:, s * ph_step:end])
            out_sb = singles.tile([128, RT], mybir.dt.float32)

            for r in range(RT):
                lhsT = em_sb[:, r * 128:(r + 1) * 128]
                acc = accp.tile([128, 4], mybir.dt.float32, tag="acc")
                # First 3 psum pairs (chunks 0..5) -> ScalarE copies to SBUF.
                cp_tiles = []
                for k in range(3):
                    pt = pp.tile([128, 1024], mybir.dt.float32, tag="mm", bufs=3)
                    base = k * 1024
                    nc.tensor.matmul(pt[:, 0:512], lhsT,
                                     ph_sb[:, base:base + 512],
                                     start=True, stop=True)
                    nc.tensor.matmul(pt[:, 512:1024], lhsT,
                                     ph_sb[:, base + 512:base + 1024],
                                     start=True, stop=True)
                    cp = cps.tile([128, 1024], mybir.dt.float32, tag="cp", bufs=6)
                    nc.scalar.copy(out=cp, in_=pt)
                    cp_tiles.append(cp)
                # Next 3 psum pairs (chunks 6..11) -> fused max-max TTR against
                # the SBUF copies (ingests 2048 candidates per op).
                for k in range(3):
                    pt = pp.tile([128, 1024], mybir.dt.float32, tag="mm", bufs=3)
                    base = 3072 + k * 1024
                    nc.tensor.matmul(pt[:, 0:512], lhsT,
                                     ph_sb[:, base:base + 512],
                                     start=True, stop=True)
                    nc.tensor.matmul(pt[:, 512:1024], lhsT,
                                     ph_sb[:, base + 512:base + 1024],
                                     start=True, stop=True)
                    tscr = scr.tile([128, 1024], mybir.dt.float32,
                                    tag="scr", bufs=2)
                    nc.vector._custom_dve(
                        mm_op, out=tscr, in0=pt, in1=cp_tiles[k],
                        s0=NEG_INIT, accum_out=acc[:, k:k + 1])
                # Tail chunk (128 cols).
                tp = pp.tile([128, TAIL], mybir.dt.float32, tag="tail", bufs=2)
                nc.tensor.matmul(tp, lhsT, ph_sb[:, N_FULL * 512:PC],
                                 start=True, stop=True)
                nc.vector.reduce_max(acc[:, 3:4], tp, axis=mybir.AxisListType.X)
                nc.vector.reduce_max(out_sb[:, r:r + 1], acc,
                                     axis=mybir.AxisListType.X)

            nc.sync.dma_start(out=out[:, :], in_=out_sb)
    nc.compile()
    return nc


def _prep(purch_embeddings, pos_embs, neg_embs):
    purch = np.asarray(purch_embeddings, dtype=np.float32)
    pos = np.asarray(pos_embs, dtype=np.float32)
    neg = np.asarray(neg_embs, dtype=np.float32)

    pnorm = np.sqrt((purch.astype(np.float64) ** 2).sum(axis=1))
    phat = purch / np.maximum(pnorm, 1e-8)[:, None]
    phatT = np.zeros((128, N_CORES * PC), dtype=np.float16)
    phatT[:, :P] = phat.T.astype(np.float16)
    shards = [np.ascontiguousarray(phatT[:, c * PC:(c + 1) * PC])
              for c in range(N_CORES)]

    embs = np.concatenate(
        [pos.reshape(B * NPOS, D), neg.reshape(B * NNEG, D)], axis=0)
    enorm = np.sqrt((embs.astype(np.float64) ** 2).sum(axis=1))
    embsT = np.ascontiguousarray(embs.T.astype(np.float16))
    return shards, embsT, enorm


def run_device(shards, embsT, trace=False):
    if "nc" not in _CACHE:
        _CACHE["nc"] = build_nc()
    nc = _CACHE["nc"]
    in_maps = [{"ph": shards[c], "em": embsT} for c in range(N_CORES)]
    kwargs = {}
    if trace:
        kwargs = dict(trace=True, tmpdir=tempfile.mkdtemp(prefix="ctr_"))
    return run_bass_kernel_spmd(nc, in_maps, core_ids=list(range(N_CORES)),
                                **kwargs)


def _finish(results, enorm, cost_pos, cost_neg, neg_seg_ids):
    # Gather/unshard: global max over cores, then the tiny per-sample loss.
    parts = np.stack([r["out"].T.reshape(ROWS) for r in results])  # [8, 4608]
    M = parts.max(axis=0).astype(np.float64)                       # [4608]

    cos_max = M / np.maximum(enorm, 1e-8)
    min_dist = 1.0 - cos_max
    pos_min = min_dist[:B * NPOS].reshape(B, NPOS)
    neg_min = min_dist[B * NPOS:].reshape(B, NNEG)

    cost_pos = np.asarray(cost_pos, dtype=np.float64)
    cost_neg = np.asarray(cost_neg, dtype=np.float64)
    seg = np.asarray(neg_seg_ids).astype(np.int64)

    positive_value = pos_min.sum(axis=1) + cost_pos                # [B]
    seg_sum = np.zeros((B, NUM_GROUPS), dtype=np.float64)
    np.add.at(seg_sum, (np.arange(B)[:, None], seg), neg_min)
    negative_values = seg_sum + cost_neg                           # [B, G]

    num = np.exp(-positive_value)
    den = np.exp(-negative_values).sum(axis=1)
    losses = -np.log(num / (num + den))
    return np.array(losses.mean(), dtype=np.float32)


def kernel(purch_embeddings, pos_embs, neg_embs, cost_pos, cost_neg,
           neg_seg_ids):
    shards, embsT, enorm = _prep(purch_embeddings, pos_embs, neg_embs)
    results = run_device(shards, embsT, trace=False)
    return _finish(results.results, enorm, cost_pos, cost_neg, neg_seg_ids)
